# revision 32
# baseline (speedup 1.0000x reference)
"""TRN2 Bass/Tile kernel: 16-head MHA (N=2, S=2048, D=1024) on 8 NeuronCores.

Sharding (hardcoded): core c = 4*n + g runs batch n (data parallel, N=2) and
head group g (tensor parallel, 4 heads each).  Wq/Wk/Wv are column-sharded
[1024, 256], Wp row-sharded [256, 1024].  Each core produces a partial
projection [2048, 1024] (bf16); the host sums the 4 partials of each batch
and adds the (bv @ Wp + bp) terms (exact, since dropout is identity and the
projection is linear in bv).

Device-side dataflow per core (all matmuls bf16 with fp32 PSUM accumulation):
  - host hands the core pre-transposed, pre-bf16-cast activations: xq/xk as
    x^T [1024, 2048]; xv pre-tiled by 128-column sequence chunks
  - Q^T, K^T [256, 2048] computed with heads on partitions (head pairs share
    a 128-partition chunk); V [2048, 256] per seq chunk, all upfront and
    DMA-paced; V gets a 65th ones-column per head so the A@V matmul also
    produces the softmax denominator (PSUM row 64) -- no separate
    ones-matmul pass over the exp weights
  - scores are computed transposed (keys on partitions, queries free) as
    row-tiled concurrent matmul pairs (each head contracts only HD=64, so two
    heads run in the two 64-row halves of the PE array)
  - exp: ScalarE handles 3 of the 4 [128,512] score quarters per chunk;
    VectorE handles the 4th with a bitcast 2^y construction (tensor_scalar
    into int16 viewed as bf16) unless stage flag "x0" disables it
  - A@V runs per (query-half iq, head e) into a [65, 512] PSUM tile with
    lhsT = [V_e | 1] (M=65, full 128-key contraction); row 64 accumulates
    the denominator
  - softmax normalization is deferred: O^T_unnorm accumulates over all keys,
    then rows are scaled by 1/denom (reciprocal + scalar-ring broadcast +
    multiply) before the output projection; each query-half's output
    projection runs right after its attention over the freed PSUM banks,
    with the c=0 matmuls (whose ot half was normalized a head-pair ago)
    leading so the fresh normalize chain hides behind them

PSUM budget (8 banks): four [128, 2, 512] f32 tags (sca, scb, av0, av1; 2
banks each).  Scores own sca/scb inside the attention loops; A@V owns
av0/av1 (partitions 0..64 of each half); the Q/K/V and output projections
and the V-proj staging time-share all four tags outside them.
"""

import numpy as np

N, S, D = 2, 2048, 1024
H, HD = 16, 64
NHL = 4                 # heads per core
DH = NHL * HD           # 256 local channels
P = 128
KC = D // P             # 8 contraction chunks for the projections
SC = S // P             # 16 sequence chunks
IH = S // 2             # queries per i-half
HD1 = HD + 1            # head dim + ones column (denominator row)

LOG2E_8 = 0.125 / float(np.log(2.0))   # scores -> log2 weights
BF16_MAGIC = 16256.0                   # 127 << 7

_built = {}


def _emit(tc, out, xqt, xkt, xvj, wq, wk, wv, wp, bq, bk, stage="full"):
    from concourse import mybir

    nc = tc.nc
    f32 = mybir.dt.float32
    bf16 = mybir.dt.bfloat16
    i16 = mybir.dt.int16
    Exp = mybir.ActivationFunctionType.Exp
    MUL = mybir.AluOpType.mult
    ADD = mybir.AluOpType.add

    flags = stage.split("_")
    base = flags[0]
    # by default the DVE computes one of the four score quarters per chunk
    # with a bitcast 2^y construction, consumed 4 chunks late so the write
    # never gates the PE; "x0" reverts to all-ScalarE exact exp
    dve_exp = "x0" not in flags

    with (
        tc.tile_pool(name="const", bufs=1) as cpool,
        tc.tile_pool(name="work", bufs=1) as wpool,
        tc.tile_pool(name="e", bufs=3) as epool,
        tc.tile_pool(name="small", bufs=2) as spool,
        tc.tile_pool(name="ob", bufs=3) as opool,
        tc.tile_pool(name="ps", bufs=1, space="PSUM") as ps,
    ):
        # ---------- weights / constants ----------
        wq_sb = cpool.tile([P, KC, DH], bf16)
        wk_sb = cpool.tile([P, KC, DH], bf16)
        wv_sb = cpool.tile([P, KC, DH], bf16)
        wp_sb = cpool.tile([P, 2, D], bf16)
        bq_sb = cpool.tile([P, 2], f32)
        bk_sb = cpool.tile([P, 2], f32)
        ones_sb = cpool.tile([P, 32], bf16)
        nc.vector.memset(ones_sb[:], 1.0)

        # sync (HWDGE) queue carries the critical-path loads in strict order:
        # wq, xq, wk, xk, then the xv seq-chunk tiles.  gpsimd (SWDGE) takes
        # the small non-critical weights up front and the stores later.
        nc.gpsimd.dma_start(bq_sb[:], bq.rearrange("(c p) -> p c", p=P))
        nc.gpsimd.dma_start(bk_sb[:], bk.rearrange("(c p) -> p c", p=P))
        nc.gpsimd.dma_start(wv_sb[:], wv.rearrange("(kc p) d -> p kc d", p=P))

        xq_sb = wpool.tile([P, KC, S], bf16)
        xk_sb = wpool.tile([P, KC, S], bf16)
        xv_sb = wpool.tile([P, SC, KC, P], bf16)
        # split wq so the first Q-proj matmuls wait only on the first half
        nc.sync.dma_start(wq_sb[:, 0:KC // 2, :],
                          wq[0:D // 2].rearrange("(kc p) d -> p kc d", p=P))
        nc.sync.dma_start(wq_sb[:, KC // 2:KC, :],
                          wq[D // 2:D].rearrange("(kc p) d -> p kc d", p=P))
        for kc in range(KC):
            nc.sync.dma_start(xq_sb[:, kc, :], xqt[kc * P:(kc + 1) * P, :])
        nc.sync.dma_start(wk_sb[:], wk.rearrange("(kc p) d -> p kc d", p=P))
        for kc in range(KC):
            nc.sync.dma_start(xk_sb[:, kc, :], xkt[kc * P:(kc + 1) * P, :])
        xv_eng = nc.gpsimd if "xvg" in flags else nc.sync
        for jc in range(SC):
            xv_eng.dma_start(xv_sb[:, jc, :, :], xvj[jc])
        # wp is only needed by the output projection; last on the sync ring
        nc.sync.dma_start(wp_sb[:], wp.rearrange("(c p) e -> p c e", p=P))

        def consume(slices):
            # tiny accumulating matmuls defeat dead-code elimination without
            # perturbing the DMA queues (one small store at the end)
            pc = ps.tile([P, 2, 512], f32, tag="sca", name="pc")
            for i, sl in enumerate(slices):
                nc.tensor.matmul(
                    pc[0:8, 0, 0:32], lhsT=sl, rhs=ones_sb[:, :],
                    start=(i == 0), stop=(i == len(slices) - 1),
                )
            cb = opool.tile([8, 32], bf16, tag="cb", name="cb")
            nc.vector.tensor_copy(cb[:], pc[0:8, 0, 0:32])
            nc.gpsimd.dma_start(out[0:8, 0:32], cb[:])

        if base == "load":
            consume(
                [xq_sb[:, kc, 0:8] for kc in range(KC)]
                + [xk_sb[:, kc, 0:8] for kc in range(KC)]
                + [xv_sb[:, jc, 0, 0:8] for jc in range(SC)]
            )
            return

        # ---------- Q/K/V projections (all upfront, DMA-paced) ----------
        qt_sb = wpool.tile([P, 2, S], bf16)
        kt_sb = wpool.tile([P, 2, S], bf16)
        v_sb = wpool.tile([P, SC, NHL, HD1], bf16)
        # ones column for every (chunk, head): A@V row 64 = softmax denom
        nc.vector.memset(v_sb[:, :, :, HD], 1.0)

        # All eight PSUM banks carry the same [P, 2, 512] tag shape (sca,
        # scb, av0, av1); attention A@V writes partitions 0..64 of each
        # 512-column half, everything else uses full 128-partition tiles.
        PTAGS = ("sca", "scb", "av0", "av1")

        def emit_proj(x_sb, w_sb, b_sb, dst):
            # kc-outer streaming: each xq/xk chunk arrives by DMA and is
            # immediately contracted for all four (c, icp) accumulators, so
            # the PE consumes chunks at the DMA arrival rate
            pts = {}
            for icp in range(2):
                for c in range(2):
                    pts[c, icp] = ps.tile([P, 2, 512], f32,
                                          tag=PTAGS[2 * icp + c], name="pts")
            for kc in range(KC):
                for icp in range(2):
                    for c in range(2):
                        for j in range(2):
                            ic = icp * 2 + j
                            nc.tensor.matmul(
                                pts[c, icp][:, j, :],
                                lhsT=w_sb[:, kc, c * P:(c + 1) * P],
                                rhs=x_sb[:, kc, ic * 512:(ic + 1) * 512],
                                start=(kc == 0),
                                stop=(kc == KC - 1),
                            )
            for icp in range(2):
                for c in range(2):
                    for j in range(2):
                        ic = icp * 2 + j
                        nc.vector.tensor_scalar(
                            dst[:, c, ic * 512:(ic + 1) * 512],
                            pts[c, icp][:, j, :],
                            b_sb[:, c:c + 1],
                            None,
                            ADD,
                        )

        emit_proj(xq_sb, wq_sb, bq_sb, qt_sb)
        emit_proj(xk_sb, wk_sb, bk_sb, kt_sb)

        def emit_vproj(jc):
            # rotate through all four tags so the v_sb copy of chunk jc
            # overlaps the matmuls of chunks jc+1..jc+3
            pv = ps.tile([P, 2, 512], f32, tag=PTAGS[jc % 4], name="pv")
            for kc in range(KC):
                nc.tensor.matmul(
                    pv[:, 0, 0:DH],
                    lhsT=xv_sb[:, jc, kc, :],
                    rhs=wv_sb[:, kc, :],
                    start=(kc == 0),
                    stop=(kc == KC - 1),
                )
            nc.vector.tensor_copy(
                v_sb[:, jc, :, 0:HD],
                pv[:, 0, 0:DH].rearrange("p (h d) -> p h d", d=HD),
            )

        for jc in range(SC):
            emit_vproj(jc)

        if base == "proj":
            consume(
                [qt_sb[:, c, ic * 512:ic * 512 + 8]
                 for c in range(2) for ic in range(4)]
                + [kt_sb[:, c, ic * 512:ic * 512 + 8]
                   for c in range(2) for ic in range(4)]
                + [v_sb[:, jc, 0, 0:8] for jc in range(SC)]
            )
            return

        # ---------- attention ----------
        ot = [None, None]

        def emit_outproj_quad(ih, q4):
            # four query chunks at once: all c=0 matmuls first (their ot
            # half was normalized a head-pair ago), so the c=1 matmuls give
            # the just-finished pair's normalize chain ~3.4us of PE runway
            pos = [ps.tile([P, 2, 512], f32, tag=PTAGS[i], name="po")
                   for i in range(4)]
            for c in range(2):
                for i in range(4):
                    ic8 = q4 * 4 + i
                    for eh in range(2):
                        nc.tensor.matmul(
                            pos[i][:, eh, :],
                            lhsT=ot[ih][c][:, ic8 * P:(ic8 + 1) * P],
                            rhs=wp_sb[:, c, eh * 512:(eh + 1) * 512],
                            start=(c == 0),
                            stop=(c == 1),
                        )
            for i in range(4):
                ic8 = q4 * 4 + i
                r0 = ih * IH + ic8 * P
                ob = opool.tile([P, 2, 512], out.dtype, tag="ob", name="ob",
                                bufs=5)
                # split the two PSUM->SBUF copies across ScalarE and VectorE
                nc.scalar.copy(ob[:, 0, :], pos[i][:, 0, :])
                nc.vector.tensor_copy(ob[:, 1, :], pos[i][:, 1, :])
                # one contiguous 256KB store (dst rows are full-width)
                nc.gpsimd.dma_start(out[r0:r0 + P, :],
                                    ob.rearrange("p a b -> p (a b)"))

        for ih in range(2):
            i0 = ih * IH
            ot[ih] = [wpool.tile([P, IH], bf16, tag=f"ot{ih}{c}",
                                 name=f"ot{ih}{c}") for c in range(2)]
            for pr in range(2):
                # av[iq][:, e, :] partitions 0..64: O^T_unnorm for head
                # 2*pr+e, query half iq; row 64 accumulates the softmax
                # denominator (ones column of v_sb)
                av = [ps.tile([P, 2, 512], f32, tag=f"av{iq}",
                              name=f"av{iq}") for iq in range(2)]

                def emit_quarters(batches):
                    # batches: list of (batch, iq) with batch = [(jc, et)];
                    # e0/e1 pairs stay adjacent so they co-issue on hardware
                    if "noav" in flags:
                        return
                    if ("ls" in flags and len(batches) == 2
                            and [j for j, _ in batches[0][0]]
                            == [j for j, _ in batches[1][0]]):
                        # lockstep: share each v LDWEIGHTS between the iq0
                        # and iq1 matmuls (same lhsT, different et rhs)
                        mw = HD if "av64" in flags else HD1
                        for (jc, eta_t), (_, etb_t) in zip(batches[0][0],
                                                          batches[1][0]):
                            for e in range(2):
                                for iq, et in ((0, eta_t), (1, etb_t)):
                                    nc.tensor.matmul(
                                        av[iq][0:mw, e, :],
                                        lhsT=v_sb[:, jc, 2 * pr + e, 0:mw],
                                        rhs=et[:, e, :],
                                        start=(jc == 0),
                                        stop=(jc == SC - 1),
                                    )
                        return
                    if "il" in flags and len(batches) == 2:
                        (ba, _), (bb, _) = batches
                        seq = []
                        for k in range(max(len(ba), len(bb))):
                            if k < len(ba):
                                seq.append((ba[k], 0))
                            if k < len(bb):
                                seq.append((bb[k], 1))
                        batches = [([jcet], iq) for jcet, iq in seq]
                    for batch, iq in batches:
                        for jc, et in batch:
                            for e in range(2):
                                mw = HD if "av64" in flags else HD1
                                nc.tensor.matmul(
                                    av[iq][0:mw, e, :],
                                    lhsT=v_sb[:, jc, 2 * pr + e, 0:mw],
                                    rhs=et[:, e, :],
                                    start=(jc == 0),
                                    stop=(jc == SC - 1),
                                )

                pend_a, pend_b = [], []
                for jc in range(SC):
                    sca = ps.tile([P, 2, 512], f32, tag="sca", name="sca")
                    scb = ps.tile([P, 2, 512], f32, tag="scb", name="scb")
                    if "nosc" not in flags:
                        # zigzag: row-tile pairs stay adjacent (co-issue
                        # needs adjacent MMs in different tile positions)
                        # while the middle two share one kt LDWEIGHTS;
                        # "eo" = e-outer order: one LDWEIGHTS per head but
                        # no co-issue adjacency
                        sc_order = ((0, 0), (0, 1), (1, 1), (1, 0)) \
                            if "eo" in flags else \
                            ((0, 0), (1, 0), (1, 1), (0, 1))
                        for e, iq in sc_order:
                            sc_t = sca if iq == 0 else scb
                            nc.tensor.matmul(
                                sc_t[:, e, :],
                                lhsT=kt_sb[HD * e:HD * (e + 1), pr,
                                           jc * P:(jc + 1) * P],
                                rhs=qt_sb[HD * e:HD * (e + 1), pr,
                                          i0 + iq * 512:
                                          i0 + (iq + 1) * 512],
                                start=True,
                                stop=True,
                            )
                    eta = epool.tile([P, 2, 512], bf16, tag="eta", name="eta",
                                     bufs=4)
                    etb = epool.tile([P, 2, 512], bf16, tag="etb", name="etb",
                                     bufs=10)
                    if "noexp" in flags and "noav" not in flags:
                        nc.vector.memset(eta[:, :, 0:8], 1.0)
                        nc.vector.memset(etb[:, :, 0:8], 1.0)
                    if "noexp" not in flags:
                        if dve_exp:
                            nc.scalar.activation(etb[:, 0, :], scb[:, 0, :],
                                                 Exp, scale=0.125)
                            nc.vector.tensor_scalar(
                                etb[:, 1, :].bitcast(i16),
                                scb[:, 1, :],
                                LOG2E_8 * 128.0,
                                BF16_MAGIC,
                                MUL,
                                ADD,
                            )
                            nc.scalar.activation(eta[:], sca[:], Exp,
                                                 scale=0.125)
                        else:
                            nc.scalar.activation(eta[:], sca[:], Exp,
                                                 scale=0.125)
                            nc.scalar.activation(etb[:], scb[:], Exp,
                                                 scale=0.125)
                    pend_a.append((jc, eta))
                    pend_b.append((jc, etb))
                    nb = 4 if "b4" in flags else (1 if "b1" in flags else 2)
                    # jc >= 3: score-only runway at the head-pair boundary
                    # so the previous pair's normalize never gates the PE
                    if jc % nb == nb - 1 and 3 <= jc < SC - 1:
                        ready = [(pend_a, 0)]
                        if "ls" in flags:
                            # lockstep: both query halves consume the same
                            # chunks so the A@V pairs can share LDWEIGHTS
                            ready.append((pend_b[:len(pend_a)], 1))
                            pend_b = pend_b[len(pend_a):]
                            pend_a = []
                            emit_quarters(ready)
                            continue
                        pend_a = []
                        # the iq1 quarters run (nb+2) chunks behind when the
                        # DVE computes one of them, so its write never gates
                        # the PE
                        blag = nb + 6 if dve_exp else nb
                        if len(pend_b) >= blag:
                            ready.append((pend_b[:nb], 1))
                            pend_b = pend_b[nb:]
                        emit_quarters(ready)
                rbs = {}

                def norm_recip(iq):
                    # den from av row 64, replicated via a scalar-ring
                    # broadcast (the sync ring carries the next iteration's
                    # input loads in steady state; must not queue behind)
                    rec1 = spool.tile([P, 512], f32, tag=f"rec{iq}",
                                      name="rec1")
                    for e in range(2):
                        # engine writes start at a 32-aligned partition
                        nc.vector.reciprocal(rec1[64 * e:64 * e + 1, :],
                                             av[iq][HD:HD1, e, :])
                    rb = spool.tile([P, 512], f32, tag=f"rb{iq}", name="rb")
                    for e in range(2):
                        nc.scalar.dma_start(
                            rb[HD * e:HD * (e + 1), :],
                            rec1[64 * e:64 * e + 1, None, :].to_broadcast(
                                (1, HD, 512)),
                        )
                    rbs[iq] = rb

                def norm_mul(iq):
                    for e in range(2):
                        nc.vector.tensor_tensor(
                            ot[ih][pr][HD * e:HD * (e + 1),
                                       iq * 512:(iq + 1) * 512],
                            av[iq][0:HD, e, :],
                            rbs[iq][HD * e:HD * (e + 1), :],
                            MUL,
                        )

                if "noav" in flags:
                    emit_quarters([(pend_a, 0), (pend_b, 1)])
                    nc.vector.memset(ot[ih][pr][:, :], 0.0)
                    continue
                # flush iq0 and start its normalize while iq1's backlog
                # runs on the PE; keep both TTs behind both recips so the
                # in-order DVE queue never stalls a recip on a broadcast
                emit_quarters([(pend_a, 0)])
                norm_recip(0)
                emit_quarters([(pend_b, 1)])
                pend_b = []
                norm_recip(1)
                norm_mul(0)
                norm_mul(1)

            if base != "attn":
                # this half's output projection, over the freed PSUM banks
                for q4 in range(2):
                    emit_outproj_quad(ih, q4)

        if base == "attn":
            for ih in range(2):
                nc.gpsimd.dma_start(out[ih * P:(ih + 1) * P, :],
                                    ot[ih][0][:, :])
            return


def _build(reps=1, stage="full"):
    key = ("nc", reps, stage)
    if key in _built:
        return _built[key]
    import concourse.tile as tile
    from concourse import bacc, mybir

    f32 = mybir.dt.float32
    bf16 = mybir.dt.bfloat16
    nc = bacc.Bacc(
        "TRN2",
        target_bir_lowering=False,
        debug=False,
        num_devices=8,
    )
    xqt = nc.dram_tensor("xqt", [D, S], bf16, kind="ExternalInput").ap()
    xkt = nc.dram_tensor("xkt", [D, S], bf16, kind="ExternalInput").ap()
    xvj = nc.dram_tensor("xvj", [SC, P, D], bf16, kind="ExternalInput").ap()
    wq = nc.dram_tensor("wq", [D, DH], bf16, kind="ExternalInput").ap()
    wk = nc.dram_tensor("wk", [D, DH], bf16, kind="ExternalInput").ap()
    wv = nc.dram_tensor("wv", [D, DH], bf16, kind="ExternalInput").ap()
    wp = nc.dram_tensor("wp", [DH, D], bf16, kind="ExternalInput").ap()
    bq = nc.dram_tensor("bq", [DH], f32, kind="ExternalInput").ap()
    bk = nc.dram_tensor("bk", [DH], f32, kind="ExternalInput").ap()
    out = nc.dram_tensor("out", [S, D], bf16, kind="ExternalOutput").ap()

    with tile.TileContext(nc) as tc:
        if reps == 1:
            _emit(tc, out, xqt, xkt, xvj, wq, wk, wv, wp, bq, bk, stage=stage)
        else:
            with tc.For_i(0, reps, 1):
                _emit(tc, out, xqt, xkt, xvj, wq, wk, wv, wp, bq, bk,
                      stage=stage)
    nc.compile()
    _built[key] = nc
    return nc


def _in_maps(query, key, value, Wq, bq, Wk, bk, Wv, bv, Wp, bp):
    import ml_dtypes
    bf = ml_dtypes.bfloat16
    f = np.float32
    maps = []
    xt = {}
    for n in range(N):
        xqt = np.ascontiguousarray(np.asarray(query, f)[n].T).astype(bf)
        xkt = np.ascontiguousarray(np.asarray(key, f)[n].T).astype(bf)
        xvt = np.ascontiguousarray(np.asarray(value, f)[n].T)
        # [D, S] -> [SC, P(d-chunk), KC, 128] seq-chunk tiles
        xvj = np.ascontiguousarray(
            xvt.reshape(KC, P, SC, P).transpose(2, 1, 0, 3).reshape(SC, P, D)
        ).astype(bf)
        xt[n] = (xqt, xkt, xvj)
    for c in range(8):
        n, g = divmod(c, 4)
        lo, hi = g * DH, (g + 1) * DH
        maps.append({
            "xqt": xt[n][0],
            "xkt": xt[n][1],
            "xvj": xt[n][2],
            "wq": np.ascontiguousarray(np.asarray(Wq, f)[:, lo:hi]).astype(bf),
            "wk": np.ascontiguousarray(np.asarray(Wk, f)[:, lo:hi]).astype(bf),
            "wv": np.ascontiguousarray(np.asarray(Wv, f)[:, lo:hi]).astype(bf),
            "wp": np.ascontiguousarray(np.asarray(Wp, f)[lo:hi, :]).astype(bf),
            "bq": np.ascontiguousarray(np.asarray(bq, f)[lo:hi]),
            "bk": np.ascontiguousarray(np.asarray(bk, f)[lo:hi]),
        })
    return maps


last_results = None  # BassKernelResults of the most recent run (for test.py)


def kernel(query, key, value, Wq, bq, Wk, bk, Wv, bv, Wp, bp, trace=False,
           stage="full"):
    global last_results
    from concourse import bass_utils

    nc = _build(stage=stage)
    maps = _in_maps(query, key, value, Wq, bq, Wk, bk, Wv, bv, Wp, bp)
    res = bass_utils.run_bass_kernel_spmd(
        nc, maps, core_ids=list(range(8)), trace=trace
    )
    last_results = res

    out = np.empty((N, S, D), np.float32)
    bvp = np.asarray(bv, np.float64) @ np.asarray(Wp, np.float64)
    for n in range(N):
        acc = np.zeros((S, D), np.float64)
        for g in range(4):
            acc += res.results[4 * n + g]["out"].astype(np.float64)
        acc += bvp + np.asarray(bp, np.float64)
        out[n] = acc.astype(np.float32)
    return out


# revision 33
# speedup vs baseline: 1.0072x; 1.0072x over previous
"""TRN2 Bass/Tile kernel: 16-head MHA (N=2, S=2048, D=1024) on 8 NeuronCores.

Sharding (hardcoded): core c = 4*n + g runs batch n (data parallel, N=2) and
head group g (tensor parallel, 4 heads each).  Wq/Wk/Wv are column-sharded
[1024, 256], Wp row-sharded [256, 1024].  Each core produces a partial
projection [2048, 1024] (bf16); the host sums the 4 partials of each batch
and adds the (bv @ Wp + bp) terms (exact, since dropout is identity and the
projection is linear in bv).

Device-side dataflow per core (all matmuls bf16 with fp32 PSUM accumulation):
  - host hands the core pre-transposed, pre-bf16-cast activations: xq/xk as
    x^T [1024, 2048]; xv pre-tiled by 128-column sequence chunks
  - Q^T, K^T [256, 2048] computed with heads on partitions (head pairs share
    a 128-partition chunk); V [2048, 256] per seq chunk, all upfront and
    DMA-paced; V gets a 65th ones-column per head so the A@V matmul also
    produces the softmax denominator (PSUM row 64) -- no separate
    ones-matmul pass over the exp weights
  - scores are computed transposed (keys on partitions, queries free) as
    row-tiled concurrent matmul pairs (each head contracts only HD=64, so two
    heads run in the two 64-row halves of the PE array)
  - exp: ScalarE handles 3 of the 4 [128,512] score quarters per chunk;
    VectorE handles the 4th with a bitcast 2^y construction (tensor_scalar
    into int16 viewed as bf16) unless stage flag "x0" disables it
  - A@V runs per (query-half iq, head e) into a [65, 512] PSUM tile with
    lhsT = [V_e | 1] (M=65, full 128-key contraction); row 64 accumulates
    the denominator
  - softmax normalization is deferred: O^T_unnorm accumulates over all keys,
    then rows are scaled by 1/denom (reciprocal + scalar-ring broadcast +
    multiply) before the output projection; each query-half's output
    projection runs right after its attention over the freed PSUM banks,
    with the c=0 matmuls (whose ot half was normalized a head-pair ago)
    leading so the fresh normalize chain hides behind them

PSUM budget (8 banks): four [128, 2, 512] f32 tags (sca, scb, av0, av1; 2
banks each).  Scores own sca/scb inside the attention loops; A@V owns
av0/av1 (partitions 0..64 of each half); the Q/K/V and output projections
and the V-proj staging time-share all four tags outside them.
"""

import numpy as np

N, S, D = 2, 2048, 1024
H, HD = 16, 64
NHL = 4                 # heads per core
DH = NHL * HD           # 256 local channels
P = 128
KC = D // P             # 8 contraction chunks for the projections
SC = S // P             # 16 sequence chunks
IH = S // 2             # queries per i-half
HD1 = HD + 1            # head dim + ones column (denominator row)

LOG2E_8 = 0.125 / float(np.log(2.0))   # scores -> log2 weights
BF16_MAGIC = 16256.0                   # 127 << 7

_built = {}


def _emit(tc, out, xqt, xkt, xvj, wq, wk, wv, wp, bq, bk, stage="full"):
    from concourse import mybir

    nc = tc.nc
    f32 = mybir.dt.float32
    bf16 = mybir.dt.bfloat16
    i16 = mybir.dt.int16
    Exp = mybir.ActivationFunctionType.Exp
    MUL = mybir.AluOpType.mult
    ADD = mybir.AluOpType.add

    flags = stage.split("_")
    base = flags[0]
    # by default the DVE computes one of the four score quarters per chunk
    # with a bitcast 2^y construction, consumed 4 chunks late so the write
    # never gates the PE; "x0" reverts to all-ScalarE exact exp
    dve_exp = "x0" not in flags

    with (
        tc.tile_pool(name="const", bufs=1) as cpool,
        tc.tile_pool(name="work", bufs=1) as wpool,
        tc.tile_pool(name="e", bufs=3) as epool,
        tc.tile_pool(name="small", bufs=2) as spool,
        tc.tile_pool(name="ob", bufs=3) as opool,
        tc.tile_pool(name="ps", bufs=1, space="PSUM") as ps,
    ):
        # ---------- weights / constants ----------
        wq_sb = cpool.tile([P, KC, DH], bf16)
        wk_sb = cpool.tile([P, KC, DH], bf16)
        wv_sb = cpool.tile([P, KC, DH], bf16)
        wp_sb = cpool.tile([P, 2, D], bf16)
        bq_sb = cpool.tile([P, 2], f32)
        bk_sb = cpool.tile([P, 2], f32)
        ones_sb = cpool.tile([P, 32], bf16)
        nc.vector.memset(ones_sb[:], 1.0)

        # sync (HWDGE) queue carries the critical-path loads in strict order:
        # wq, xq, wk, xk, then the xv seq-chunk tiles.  gpsimd (SWDGE) takes
        # the small non-critical weights up front and the stores later.
        nc.gpsimd.dma_start(bq_sb[:], bq.rearrange("(c p) -> p c", p=P))
        nc.gpsimd.dma_start(bk_sb[:], bk.rearrange("(c p) -> p c", p=P))
        nc.gpsimd.dma_start(wv_sb[:], wv.rearrange("(kc p) d -> p kc d", p=P))

        xq_sb = wpool.tile([P, KC, S], bf16)
        xk_sb = wpool.tile([P, KC, S], bf16)
        xv_sb = wpool.tile([P, SC, KC, P], bf16)
        # split wq so the first Q-proj matmuls wait only on the first half
        nc.sync.dma_start(wq_sb[:, 0:KC // 2, :],
                          wq[0:D // 2].rearrange("(kc p) d -> p kc d", p=P))
        nc.sync.dma_start(wq_sb[:, KC // 2:KC, :],
                          wq[D // 2:D].rearrange("(kc p) d -> p kc d", p=P))
        for kc in range(KC):
            nc.sync.dma_start(xq_sb[:, kc, :], xqt[kc * P:(kc + 1) * P, :])
        nc.sync.dma_start(wk_sb[:], wk.rearrange("(kc p) d -> p kc d", p=P))
        for kc in range(KC):
            nc.sync.dma_start(xk_sb[:, kc, :], xkt[kc * P:(kc + 1) * P, :])
        xv_eng = nc.gpsimd if "xvg" in flags else nc.sync
        for jc in range(SC):
            xv_eng.dma_start(xv_sb[:, jc, :, :], xvj[jc])
        # wp is only needed by the output projection; last on the sync ring
        nc.sync.dma_start(wp_sb[:], wp.rearrange("(c p) e -> p c e", p=P))

        def consume(slices):
            # tiny accumulating matmuls defeat dead-code elimination without
            # perturbing the DMA queues (one small store at the end)
            pc = ps.tile([P, 2, 512], f32, tag="sca", name="pc")
            for i, sl in enumerate(slices):
                nc.tensor.matmul(
                    pc[0:8, 0, 0:32], lhsT=sl, rhs=ones_sb[:, :],
                    start=(i == 0), stop=(i == len(slices) - 1),
                )
            cb = opool.tile([8, 32], bf16, tag="cb", name="cb")
            nc.vector.tensor_copy(cb[:], pc[0:8, 0, 0:32])
            nc.gpsimd.dma_start(out[0:8, 0:32], cb[:])

        if base == "load":
            consume(
                [xq_sb[:, kc, 0:8] for kc in range(KC)]
                + [xk_sb[:, kc, 0:8] for kc in range(KC)]
                + [xv_sb[:, jc, 0, 0:8] for jc in range(SC)]
            )
            return

        # ---------- Q/K/V projections (all upfront, DMA-paced) ----------
        qt_sb = wpool.tile([P, 2, S], bf16)
        kt_sb = wpool.tile([P, 2, S], bf16)
        v_sb = wpool.tile([P, SC, NHL, HD1], bf16)
        # ones column for every (chunk, head): A@V row 64 = softmax denom
        nc.vector.memset(v_sb[:, :, :, HD], 1.0)

        # All eight PSUM banks carry the same [P, 2, 512] tag shape (sca,
        # scb, av0, av1); attention A@V writes partitions 0..64 of each
        # 512-column half, everything else uses full 128-partition tiles.
        PTAGS = ("sca", "scb", "av0", "av1")

        def emit_proj(x_sb, w_sb, b_sb, dst):
            # kc-outer streaming: each xq/xk chunk arrives by DMA and is
            # immediately contracted for all four (c, icp) accumulators, so
            # the PE consumes chunks at the DMA arrival rate
            pts = {}
            for icp in range(2):
                for c in range(2):
                    pts[c, icp] = ps.tile([P, 2, 512], f32,
                                          tag=PTAGS[2 * icp + c], name="pts")
            for kc in range(KC):
                for icp in range(2):
                    for c in range(2):
                        for j in range(2):
                            ic = icp * 2 + j
                            nc.tensor.matmul(
                                pts[c, icp][:, j, :],
                                lhsT=w_sb[:, kc, c * P:(c + 1) * P],
                                rhs=x_sb[:, kc, ic * 512:(ic + 1) * 512],
                                start=(kc == 0),
                                stop=(kc == KC - 1),
                            )
            for icp in range(2):
                for c in range(2):
                    for j in range(2):
                        ic = icp * 2 + j
                        nc.vector.tensor_scalar(
                            dst[:, c, ic * 512:(ic + 1) * 512],
                            pts[c, icp][:, j, :],
                            b_sb[:, c:c + 1],
                            None,
                            ADD,
                        )

        emit_proj(xq_sb, wq_sb, bq_sb, qt_sb)
        emit_proj(xk_sb, wk_sb, bk_sb, kt_sb)

        def emit_vproj(jc):
            # rotate through all four tags so the v_sb copy of chunk jc
            # overlaps the matmuls of chunks jc+1..jc+3
            pv = ps.tile([P, 2, 512], f32, tag=PTAGS[jc % 4], name="pv")
            for kc in range(KC):
                nc.tensor.matmul(
                    pv[:, 0, 0:DH],
                    lhsT=xv_sb[:, jc, kc, :],
                    rhs=wv_sb[:, kc, :],
                    start=(kc == 0),
                    stop=(kc == KC - 1),
                )
            nc.vector.tensor_copy(
                v_sb[:, jc, :, 0:HD],
                pv[:, 0, 0:DH].rearrange("p (h d) -> p h d", d=HD),
            )

        for jc in range(SC):
            emit_vproj(jc)

        if base == "proj":
            consume(
                [qt_sb[:, c, ic * 512:ic * 512 + 8]
                 for c in range(2) for ic in range(4)]
                + [kt_sb[:, c, ic * 512:ic * 512 + 8]
                   for c in range(2) for ic in range(4)]
                + [v_sb[:, jc, 0, 0:8] for jc in range(SC)]
            )
            return

        # ---------- attention ----------
        ot = [None, None]

        def emit_outproj_quad(ih, q4):
            # four query chunks at once: all c=0 matmuls first (their ot
            # half was normalized a head-pair ago), so the c=1 matmuls give
            # the just-finished pair's normalize chain ~3.4us of PE runway
            pos = [ps.tile([P, 2, 512], f32, tag=PTAGS[i], name="po")
                   for i in range(4)]
            for c in range(2):
                for i in range(4):
                    ic8 = q4 * 4 + i
                    for eh in range(2):
                        nc.tensor.matmul(
                            pos[i][:, eh, :],
                            lhsT=ot[ih][c][:, ic8 * P:(ic8 + 1) * P],
                            rhs=wp_sb[:, c, eh * 512:(eh + 1) * 512],
                            start=(c == 0),
                            stop=(c == 1),
                        )
            for i in range(4):
                ic8 = q4 * 4 + i
                r0 = ih * IH + ic8 * P
                ob = opool.tile([P, 2, 512], out.dtype, tag="ob", name="ob",
                                bufs=5)
                # split the two PSUM->SBUF copies across ScalarE and VectorE
                nc.scalar.copy(ob[:, 0, :], pos[i][:, 0, :])
                nc.vector.tensor_copy(ob[:, 1, :], pos[i][:, 1, :])
                # one contiguous 256KB store (dst rows are full-width)
                nc.gpsimd.dma_start(out[r0:r0 + P, :],
                                    ob.rearrange("p a b -> p (a b)"))

        for ih in range(2):
            i0 = ih * IH
            ot[ih] = [wpool.tile([P, IH], bf16, tag=f"ot{ih}{c}",
                                 name=f"ot{ih}{c}") for c in range(2)]
            for pr in range(2):
                # av[iq][:, e, :] partitions 0..64: O^T_unnorm for head
                # 2*pr+e, query half iq; row 64 accumulates the softmax
                # denominator (ones column of v_sb)
                av = [ps.tile([P, 2, 512], f32, tag=f"av{iq}",
                              name=f"av{iq}") for iq in range(2)]

                def emit_quarters(batches):
                    # batches: list of (batch, iq) with batch = [(jc, et)];
                    # e0/e1 pairs stay adjacent so they co-issue on hardware
                    if "noav" in flags:
                        return
                    if ("ls" in flags and len(batches) == 2
                            and [j for j, _ in batches[0][0]]
                            == [j for j, _ in batches[1][0]]):
                        # lockstep: share each v LDWEIGHTS between the iq0
                        # and iq1 matmuls (same lhsT, different et rhs)
                        mw = HD if "av64" in flags else HD1
                        for (jc, eta_t), (_, etb_t) in zip(batches[0][0],
                                                          batches[1][0]):
                            for e in range(2):
                                for iq, et in ((0, eta_t), (1, etb_t)):
                                    nc.tensor.matmul(
                                        av[iq][0:mw, e, :],
                                        lhsT=v_sb[:, jc, 2 * pr + e, 0:mw],
                                        rhs=et[:, e, :],
                                        start=(jc == 0),
                                        stop=(jc == SC - 1),
                                    )
                        return
                    if "il" in flags and len(batches) == 2:
                        (ba, _), (bb, _) = batches
                        seq = []
                        for k in range(max(len(ba), len(bb))):
                            if k < len(ba):
                                seq.append((ba[k], 0))
                            if k < len(bb):
                                seq.append((bb[k], 1))
                        batches = [([jcet], iq) for jcet, iq in seq]
                    for batch, iq in batches:
                        for jc, et in batch:
                            for e in range(2):
                                mw = HD if "av64" in flags else HD1
                                nc.tensor.matmul(
                                    av[iq][0:mw, e, :],
                                    lhsT=v_sb[:, jc, 2 * pr + e, 0:mw],
                                    rhs=et[:, e, :],
                                    start=(jc == 0),
                                    stop=(jc == SC - 1),
                                )

                pend_a, pend_b = [], []
                for jc in range(SC):
                    if "sb16" in flags:
                        # bf16 scores: one [P,2,1024] tile (same bytes as
                        # the f32 tags) holds all 4 quarters of a chunk, so
                        # consecutive chunks alternate sca/scb and the
                        # exp WAR period doubles (PE pstate stays ramped)
                        sc16 = ps.tile([P, 2, 1024], bf16,
                                       tag=("sca" if jc % 2 == 0 else "scb"),
                                       name="sc16")
                        sca = sc16[:, :, 0:512]
                        scb = sc16[:, :, 512:1024]
                    else:
                        sca = ps.tile([P, 2, 512], f32, tag="sca", name="sca")
                        scb = ps.tile([P, 2, 512], f32, tag="scb", name="scb")
                    if "nosc" not in flags:
                        # zigzag: row-tile pairs stay adjacent (co-issue
                        # needs adjacent MMs in different tile positions)
                        # while the middle two share one kt LDWEIGHTS;
                        # "eo" = e-outer order: one LDWEIGHTS per head but
                        # no co-issue adjacency
                        sc_order = ((0, 0), (0, 1), (1, 1), (1, 0)) \
                            if "eo" in flags else \
                            ((0, 0), (1, 0), (1, 1), (0, 1))
                        for e, iq in sc_order:
                            sc_t = sca if iq == 0 else scb
                            nc.tensor.matmul(
                                sc_t[:, e, :],
                                lhsT=kt_sb[HD * e:HD * (e + 1), pr,
                                           jc * P:(jc + 1) * P],
                                rhs=qt_sb[HD * e:HD * (e + 1), pr,
                                          i0 + iq * 512:
                                          i0 + (iq + 1) * 512],
                                start=True,
                                stop=True,
                            )
                    eta = epool.tile([P, 2, 512], bf16, tag="eta", name="eta",
                                     bufs=4)
                    etb = epool.tile([P, 2, 512], bf16, tag="etb", name="etb",
                                     bufs=10)
                    if "noexp" in flags and "noav" not in flags:
                        nc.vector.memset(eta[:, :, 0:8], 1.0)
                        nc.vector.memset(etb[:, :, 0:8], 1.0)
                    if "noexp" not in flags:
                        if dve_exp:
                            nc.scalar.activation(etb[:, 0, :], scb[:, 0, :],
                                                 Exp, scale=0.125)
                            nc.vector.tensor_scalar(
                                etb[:, 1, :].bitcast(i16),
                                scb[:, 1, :],
                                LOG2E_8 * 128.0,
                                BF16_MAGIC,
                                MUL,
                                ADD,
                            )
                            nc.scalar.activation(eta[:], sca[:], Exp,
                                                 scale=0.125)
                        else:
                            nc.scalar.activation(eta[:], sca[:], Exp,
                                                 scale=0.125)
                            nc.scalar.activation(etb[:], scb[:], Exp,
                                                 scale=0.125)
                    pend_a.append((jc, eta))
                    pend_b.append((jc, etb))
                    nb = 4 if "b4" in flags else (1 if "b1" in flags else 2)
                    # jc >= 3: score-only runway at the head-pair boundary
                    # so the previous pair's normalize never gates the PE
                    if jc % nb == nb - 1 and 3 <= jc < SC - 1:
                        ready = [(pend_a, 0)]
                        if "ls" in flags:
                            # lockstep: both query halves consume the same
                            # chunks so the A@V pairs can share LDWEIGHTS
                            ready.append((pend_b[:len(pend_a)], 1))
                            pend_b = pend_b[len(pend_a):]
                            pend_a = []
                            emit_quarters(ready)
                            continue
                        pend_a = []
                        # the iq1 quarters run (nb+2) chunks behind when the
                        # DVE computes one of them, so its write never gates
                        # the PE
                        blag = nb + 6 if dve_exp else nb
                        if len(pend_b) >= blag:
                            ready.append((pend_b[:nb], 1))
                            pend_b = pend_b[nb:]
                        emit_quarters(ready)
                rbs = {}

                def norm_recip(iq):
                    # den from av row 64, replicated via a scalar-ring
                    # broadcast (the sync ring carries the next iteration's
                    # input loads in steady state; must not queue behind)
                    rec1 = spool.tile([P, 512], f32, tag=f"rec{iq}",
                                      name="rec1")
                    for e in range(2):
                        # engine writes start at a 32-aligned partition
                        nc.vector.reciprocal(rec1[64 * e:64 * e + 1, :],
                                             av[iq][HD:HD1, e, :])
                    rb = spool.tile([P, 512], f32, tag=f"rb{iq}", name="rb")
                    for e in range(2):
                        nc.scalar.dma_start(
                            rb[HD * e:HD * (e + 1), :],
                            rec1[64 * e:64 * e + 1, None, :].to_broadcast(
                                (1, HD, 512)),
                        )
                    rbs[iq] = rb

                def norm_mul(iq):
                    for e in range(2):
                        nc.vector.tensor_tensor(
                            ot[ih][pr][HD * e:HD * (e + 1),
                                       iq * 512:(iq + 1) * 512],
                            av[iq][0:HD, e, :],
                            rbs[iq][HD * e:HD * (e + 1), :],
                            MUL,
                        )

                if "noav" in flags:
                    emit_quarters([(pend_a, 0), (pend_b, 1)])
                    nc.vector.memset(ot[ih][pr][:, :], 0.0)
                    continue
                # flush iq0 and start its normalize while iq1's backlog
                # runs on the PE; keep both TTs behind both recips so the
                # in-order DVE queue never stalls a recip on a broadcast
                emit_quarters([(pend_a, 0)])
                norm_recip(0)
                emit_quarters([(pend_b, 1)])
                pend_b = []
                norm_recip(1)
                norm_mul(0)
                norm_mul(1)

            if base != "attn":
                # this half's output projection, over the freed PSUM banks
                for q4 in range(2):
                    emit_outproj_quad(ih, q4)

        if base == "attn":
            for ih in range(2):
                nc.gpsimd.dma_start(out[ih * P:(ih + 1) * P, :],
                                    ot[ih][0][:, :])
            return


def _build(reps=1, stage="full"):
    key = ("nc", reps, stage)
    if key in _built:
        return _built[key]
    import concourse.tile as tile
    from concourse import bacc, mybir

    f32 = mybir.dt.float32
    bf16 = mybir.dt.bfloat16
    nc = bacc.Bacc(
        "TRN2",
        target_bir_lowering=False,
        debug=False,
        num_devices=8,
    )
    xqt = nc.dram_tensor("xqt", [D, S], bf16, kind="ExternalInput").ap()
    xkt = nc.dram_tensor("xkt", [D, S], bf16, kind="ExternalInput").ap()
    xvj = nc.dram_tensor("xvj", [SC, P, D], bf16, kind="ExternalInput").ap()
    wq = nc.dram_tensor("wq", [D, DH], bf16, kind="ExternalInput").ap()
    wk = nc.dram_tensor("wk", [D, DH], bf16, kind="ExternalInput").ap()
    wv = nc.dram_tensor("wv", [D, DH], bf16, kind="ExternalInput").ap()
    wp = nc.dram_tensor("wp", [DH, D], bf16, kind="ExternalInput").ap()
    bq = nc.dram_tensor("bq", [DH], f32, kind="ExternalInput").ap()
    bk = nc.dram_tensor("bk", [DH], f32, kind="ExternalInput").ap()
    out = nc.dram_tensor("out", [S, D], bf16, kind="ExternalOutput").ap()

    with tile.TileContext(nc) as tc:
        if reps == 1:
            _emit(tc, out, xqt, xkt, xvj, wq, wk, wv, wp, bq, bk, stage=stage)
        else:
            with tc.For_i(0, reps, 1):
                _emit(tc, out, xqt, xkt, xvj, wq, wk, wv, wp, bq, bk,
                      stage=stage)
    nc.compile()
    _built[key] = nc
    return nc


def _in_maps(query, key, value, Wq, bq, Wk, bk, Wv, bv, Wp, bp):
    import ml_dtypes
    bf = ml_dtypes.bfloat16
    f = np.float32
    maps = []
    xt = {}
    for n in range(N):
        xqt = np.ascontiguousarray(np.asarray(query, f)[n].T).astype(bf)
        xkt = np.ascontiguousarray(np.asarray(key, f)[n].T).astype(bf)
        xvt = np.ascontiguousarray(np.asarray(value, f)[n].T)
        # [D, S] -> [SC, P(d-chunk), KC, 128] seq-chunk tiles
        xvj = np.ascontiguousarray(
            xvt.reshape(KC, P, SC, P).transpose(2, 1, 0, 3).reshape(SC, P, D)
        ).astype(bf)
        xt[n] = (xqt, xkt, xvj)
    for c in range(8):
        n, g = divmod(c, 4)
        lo, hi = g * DH, (g + 1) * DH
        maps.append({
            "xqt": xt[n][0],
            "xkt": xt[n][1],
            "xvj": xt[n][2],
            "wq": np.ascontiguousarray(np.asarray(Wq, f)[:, lo:hi]).astype(bf),
            "wk": np.ascontiguousarray(np.asarray(Wk, f)[:, lo:hi]).astype(bf),
            "wv": np.ascontiguousarray(np.asarray(Wv, f)[:, lo:hi]).astype(bf),
            "wp": np.ascontiguousarray(np.asarray(Wp, f)[lo:hi, :]).astype(bf),
            "bq": np.ascontiguousarray(np.asarray(bq, f)[lo:hi]),
            "bk": np.ascontiguousarray(np.asarray(bk, f)[lo:hi]),
        })
    return maps


last_results = None  # BassKernelResults of the most recent run (for test.py)


def kernel(query, key, value, Wq, bq, Wk, bk, Wv, bv, Wp, bp, trace=False,
           stage="full"):
    global last_results
    from concourse import bass_utils

    nc = _build(stage=stage)
    maps = _in_maps(query, key, value, Wq, bq, Wk, bk, Wv, bv, Wp, bp)
    res = bass_utils.run_bass_kernel_spmd(
        nc, maps, core_ids=list(range(8)), trace=trace
    )
    last_results = res

    out = np.empty((N, S, D), np.float32)
    bvp = np.asarray(bv, np.float64) @ np.asarray(Wp, np.float64)
    for n in range(N):
        acc = np.zeros((S, D), np.float64)
        for g in range(4):
            acc += res.results[4 * n + g]["out"].astype(np.float64)
        acc += bvp + np.asarray(bp, np.float64)
        out[n] = acc.astype(np.float32)
    return out
